# revision 33
# baseline (speedup 1.0000x reference)
"""Trainium2 Bass kernel for nn_AdvDiffSolver: 1D advection-diffusion explicit Euler.

y_{t+1}[i] = c0*y[i] + cm*y[i-1] + cp*y[i+1]  (zero-padded boundaries), per-batch coeffs
  alpha = DT*d/DX^2, beta = DT*c/(2*DX);  c0 = 1-2a, cm = a+b, cp = a-b

FIR factorization (2 DVE ops/step): L = sigma*(1 + s1*E-)(1 + s2*E+) with
  sigma = (c0 + sqrt(c0^2 - 4*cm*cp))/2, s1 = cm/sigma, s2 = cp/sigma.
The device evolves the rescaled state within each 16-step window; a
tensor_scalar rescale by sigma^16 restores the basis at each margin refresh.
The per-window sigma^(phi+1) descale of the OUTPUT happens on the host.

Sharding: pure data parallel, 8 batches per core.  128 partitions = 16
spatial chunks x 8 batches.  Interior chunks (1..14) hold their 64-cell core
at cols [17,81) with 17-col halo margins both sides, refreshed every H=16
steps via PE shift-matmuls.  The two DOMAIN-BOUNDARY chunks are laid out
shifted so their Dirichlet pad cell falls on a column the step ops never
write: chunk 0 core at [1,65) (pad col 0; STT2 writes cols >= lo >= 1) and
chunk 15 core at [33,97) (pad col 97; STT2 writes cols < hi <= 97).  The pad
columns stay zero from init, so NO per-step boundary memsets are needed --
each step is exactly 2 fused scalar_tensor_tensor DVE ops.  Stale data
outside a boundary chunk's valid span decays inward 1 col/step and never
reaches the core within a refresh window.

Every 5 steps ONE ACT copy moves 5 states (union cols [1,97)) into the
[cell x t] accumulation buffer; each 125-step chunk is DMAd out with
per-chunk-group cell offsets so HBM only carries the 64 core cells.
Output leaves permuted [128, 64, T]; host unpermutes + descales.
"""

import numpy as np

B, N, T = 64, 1024, 1000
NCORES = 8
BL = B // NCORES      # 8 batches per core
S = 16                # spatial chunks per sample
CW = N // S           # 64 cells per chunk
M = 17                # margin cells each side (interior chunks)
H = 16                # margin refresh period (steps)
W = CW + 2 * M        # 98 tile cols
NB = 10               # state-slot rotation depth (multiple of copy group 5)
CG = 5                # steps per ACT accumulation copy
TC = 125              # time slices per accumulation chunk (8 chunks)
DX = 0.01
DT = 0.01
AC = W - 2            # 96: accumulated cols [1,97)
OFS = 3               # per-slot scalar prefix: [s1, s2, kappa]
WS = W + OFS          # 101 cols per state slot
# packed consts: init(96) | s1,s2,1.0 | s1,s2,sig16
PACK = AC + 6

# chunk s -> partition block (engine partition windows must start 32-aligned,
# so the boundary chunks sit at blocks 0 and 4: bases 0 and 32)
BLK = {}
for s in range(S):
    if s <= 3:
        BLK[s] = s
    elif s == 15:
        BLK[s] = 4
    else:
        BLK[s] = s + 1
# core column offset per chunk: boundary chunks shifted so the Dirichlet pad
# lands at col 0 (chunk 0) / col 97 (chunk 15)
C0OF = {s: (1 if s == 0 else (33 if s == 15 else M)) for s in range(S)}

_CACHE = {}


def _register_fir3():
    """Hand-authored custom DVE uop: one full Euler step per instruction.

    out[j] = kappa * (s1*y[j-1] + (1 + s1*s2)*y[j] + s2*y[j+1])
           = kappa * ((s1*y[j-1] + y[j]) + s2*(y[j+1] + s1*y[j]))

    Stream layout per row: cols [0,1,2] = s1, s2, kappa; col 3 = left pad;
    outputs at cols 4..  SRC_0 = center stream from col 0, SRC_1 = right
    stream from col 1.  The left tap is a one-element delay of SRC_0 via the
    stage-0 swap flop (BYPASS(A=CURR_SWAP_OUT, B=y_c) with swap_enable emits
    the previous element's y_c while latching the current one) --
    element-indexed state that travels with the stream, immune to issue
    bubbles.  Warm-up uops (write-suppressed) latch s1 -> swap@1+@2,
    s2 -> swap@4, kappa -> swap@7 from the first three stream elements (the
    BYPASS swap-latch stores the B operand), then one steady-shaped element
    primes the y-delay with the pad.  Outputs start at element 4.  No CONST
    operands, so there is no per-partition-scalar port penalty; kappa gives
    a free per-instruction output scale (1.0 in slots, sigma^H in yraw, so
    the refresh rebase costs nothing).
    """
    from dataclasses import dataclass
    from concourse import dve_ops as DO
    from concourse.dve_spec import Spec, Src0, Src1
    from concourse.dve_uop import (
        AluInp, AluOp, DelayInp, DveOpSpec, InpSel, OutPath, OutSel,
        Trigger, UopConfig, UopDpConfig,
    )

    for op in DO.OPS:
        if op.name == "FIR3B_ANT":
            return op

    EN, DIS = 1, 0
    A = AluInp

    def _dp(op, a, b, passthru=(), capture=None, swap=False):
        dp = UopDpConfig()
        dp.op = op
        dp.alu_src0 = a
        dp.alu_src1 = b
        dp.alu_out_enable = EN
        if swap:
            dp.swap_enable = EN
        for ln in passthru:
            dp.delay[ln] = DelayInp.PREV_DELAY
            dp.delay_enable[ln] = EN
        if capture is not None:
            dp.delay[capture] = DelayInp.PREV_ALU_OUT
            dp.delay_enable[capture] = EN
        return dp

    def _datapath():
        # lanes: d0 = y_c (SRC_0), d3 = y_r (SRC_1), d4 = m1 (captured)
        return [
            _dp(AluOp.BYPASS, A.CURR_SWAP_OUT, A.PREV_DELAY_0,
                passthru=(0, 3), swap=True),                  # y_l
            _dp(AluOp.MULTIPLY, A.PREV_ALU_OUT, A.CURR_SWAP_OUT,
                passthru=(0, 3)),                             # m1 = y_l*s1
            _dp(AluOp.MULTIPLY, A.PREV_DELAY_0, A.CURR_SWAP_OUT,
                passthru=(0, 3), capture=4),                  # m2 = y_c*s1
            _dp(AluOp.ADD, A.PREV_ALU_OUT, A.PREV_DELAY_3,
                passthru=(0, 4)),                             # a2 = m2+y_r
            _dp(AluOp.MULTIPLY, A.PREV_ALU_OUT, A.CURR_SWAP_OUT,
                passthru=(0, 4)),                             # m3 = a2*s2
            _dp(AluOp.ADD, A.PREV_ALU_OUT, A.PREV_DELAY_0,
                passthru=(4,)),                               # a3 = m3+y_c
            _dp(AluOp.ADD, A.PREV_ALU_OUT, A.PREV_DELAY_4),   # pre = a3+m1
            _dp(AluOp.MULTIPLY, A.PREV_ALU_OUT, A.CURR_SWAP_OUT),  # *kappa
        ]

    def _latch_datapath(latch_stages):
        st = [_dp(AluOp.BYPASS, A.CURR_SWAP_OUT, A.PREV_DELAY_0,
                  passthru=(0, 3), swap=True)]
        for i in range(1, 8):
            st.append(_dp(AluOp.BYPASS, A.PREV_ALU_OUT, A.PREV_DELAY_0,
                          passthru=(0, 3), swap=(i in latch_stages)))
        return st

    def _mk_uop(dp, repeat=0, nxt=(0, 0, 0), write=False):
        inp = [InpSel.ZERO] * 8
        inp_en = [DIS] * 8
        inp[1], inp_en[1] = InpSel.SRC_0, EN
        inp[4], inp_en[4] = InpSel.SRC_1, EN
        u = UopConfig()
        u.inp = inp
        u.inp_enable = inp_en
        u.require_inp0 = EN
        u.require_inp1 = EN
        u.repeat_count = repeat
        u.trigger = ((Trigger.COUNT, Trigger.NONE, Trigger.NONE) if repeat
                     else (Trigger.SRC_TENSOR_DONE, Trigger.NONE,
                           Trigger.NONE))
        u.next_uop = nxt
        u.datapath_config = dp
        if write:
            u.out_enable[OutPath.WR0_LO] = EN
            u.out[OutPath.WR0_LO] = OutSel.ALU_OUT
        else:
            for p in OutPath:
                u.out_enable[p] = DIS
        return u

    def _uops():
        return [
            _mk_uop(_latch_datapath({1, 2}), repeat=1, nxt=(1, 0, 0)),
            _mk_uop(_latch_datapath({4}), repeat=1, nxt=(2, 0, 0)),
            _mk_uop(_latch_datapath({7}), repeat=1, nxt=(3, 0, 0)),
            _mk_uop(_datapath(), repeat=1, nxt=(4, 0, 0)),
            _mk_uop(_datapath(), write=True),
        ]

    @dataclass(frozen=True)
    class HandDveOp:
        name: str
        spec: object
        subdim: bool

        def compile(self, ver):
            return DveOpSpec(
                name=self.name,
                opcode=DO.get_dve_sub_opcode(self.name),
                uops=_uops(),
                rd1_en=True,
            )

    spec = Spec(
        body=Src0 + Src1,
        reference=lambda in0, in1, s0, s1, imm2: in0 + in1,
    )
    op = HandDveOp(name="FIR3B_ANT", spec=spec, subdim=False)
    DO.OPS.append(op)
    DO._SUB_OPCODE_FOR_NAME[op.name] = DO._CUSTOM_DVE_ROW_BASE + len(DO.OPS) - 1
    DO.CUSTOM_DVE_SPECS[op.name] = spec
    return op


def _build(t_total, tc_chunk, h_refresh, m_margin):
    import concourse.bass as bass
    import concourse.bacc as bacc
    import concourse.mybir as mybir
    from concourse import tile

    dt32 = mybir.dt.float32
    w = CW + 2 * m_margin
    ws = w + OFS

    fir3 = _register_fir3()
    dt16 = mybir.dt.float16
    nc = bacc.Bacc(None, target_bir_lowering=False, debug=False)
    pack_in = nc.dram_tensor("pack", [128, PACK], dt32, kind="ExternalInput")
    packh_in = nc.dram_tensor("packh", [128, 256], dt16, kind="ExternalInput")
    # t-major output: per partition the DMA writes are fully sequential
    out_dram = nc.dram_tensor("out", [128, t_total, CW], dt32, kind="ExternalOutput")

    n_chunks = t_total // tc_chunk
    assert n_chunks * tc_chunk == t_total
    core_l = m_margin          # 17 (interior window anchor)
    core_r = m_margin + CW     # 81

    with tile.TileContext(nc) as tc:
        with (
            tc.tile_pool(name="state", bufs=1) as sp,
            tc.tile_pool(name="accp", bufs=2) as ap,
            tc.tile_pool(name="psum", bufs=2, space="PSUM") as pp,
        ):
            ybig = sp.tile([128, NB * ws], dt32, name="ybig", tag="ybig")
            yraw = sp.tile([128, ws], dt32, name="yraw", tag="yraw")
            consts = sp.tile([128, PACK], dt32, tag="consts")

            nc.sync.dma_start(consts[:], pack_in[:])
            consth = sp.tile([128, 256], dt16, tag="consth")
            nc.sync.dma_start(consth[:], packh_in[:])
            w_psl = consth[:, 0:128]
            w_psr = consth[:, 128:256]
            st = sp.tile([128, 34], dt16, tag="st")

            nc.vector.memset(ybig[:], 0.0)
            nc.vector.memset(yraw[:], 0.0)
            # init state (host pre-shifted per chunk group) into slot 0,
            # scalar triples [s1, s2, kappa] into every slot + yraw
            nc.vector.tensor_copy(ybig[:, OFS + 1:OFS + 1 + AC],
                                  consts[:, 0:AC])
            for b in range(NB):
                nc.vector.tensor_copy(ybig[:, b * ws:b * ws + 3],
                                      consts[:, AC:AC + 3])
            nc.vector.tensor_copy(yraw[:, 0:3], consts[:, AC + 3:AC + 6])
            acc = ap.tile([128, AC * tc_chunk], dt32, tag="acc")
            # time-major views: the grouped ACT copy is unit-stride on both
            # sides (slots are contiguous [nb][w] blocks; acc is [j][n])
            yv = ybig[:].rearrange("p (nb w) -> p nb w", w=ws)
            av = acc[:].rearrange("p (j n) -> p j n", n=AC)

            for t in range(1, t_total):
                cb = (t - 1) % NB
                nb = t % NB
                co = cb * ws  # cur col offset
                no = nb * ws  # nxt col offset
                cur = ybig
                if (t - 1) % h_refresh == 0:
                    psl = pp.tile([128, m_margin], dt32, tag="psl")
                    psr = pp.tile([128, m_margin], dt32, tag="psr")
                    # halo refresh: DVE stages the four source col-slices into
                    # one fp16 tile (std slice full-128, then aligned
                    # boundary-chunk overrides), then TWO fp16 partition-shift
                    # matmuls; all PSUM evacuation runs on DVE as plain
                    # copies (yraw's kappa = sigma^H does the rebase).
                    nc.vector.tensor_copy(st[:, 0:17], ybig[:, co + 67:co + 84])
                    nc.vector.tensor_copy(st[0:8, 0:17],
                                          ybig[0:8, co + 51:co + 68])
                    nc.vector.tensor_copy(st[:, 17:34], ybig[:, co + 20:co + 37])
                    nc.vector.tensor_copy(st[32:40, 17:34],
                                          ybig[32:40, co + 36:co + 53])
                    nc.tensor.matmul(psl[:], w_psl, st[:, 0:17],
                                     start=True, stop=True)
                    nc.tensor.matmul(psr[:], w_psr, st[:, 17:34],
                                     start=True, stop=True)
                    if t > 1:
                        nc.vector.tensor_copy(
                            yraw[:, 4:100], ybig[:, co + 4:co + 100])
                        # std margin moves (full 128) first, then aligned
                        # boundary-window moves, then boundary-core restores
                        # (psum rows are zero there and the std move clobbers)
                        nc.vector.tensor_copy(yraw[:, 3:20], psl[:])
                        nc.vector.tensor_copy(yraw[:, 84:101], psr[:])
                        nc.vector.tensor_copy(yraw[32:40, 19:36],
                                              psl[32:40, :])
                        nc.vector.tensor_copy(yraw[0:8, 68:85], psr[0:8, :])
                        nc.vector.tensor_copy(yraw[0:8, 4:20],
                                              ybig[0:8, co + 4:co + 20])
                        nc.vector.tensor_copy(yraw[32:40, 84:100],
                                              ybig[32:40, co + 84:co + 100])
                        cur = yraw
                        co = 0
                    else:
                        nc.vector.tensor_copy(ybig[:, 3:20], psl[:])
                        nc.vector.tensor_copy(ybig[:, 84:101], psr[:])
                        nc.vector.tensor_copy(ybig[32:40, 19:36],
                                              psl[32:40, :])
                        nc.vector.tensor_copy(ybig[0:8, 68:85], psr[0:8, :])
                        # restore boundary cores clobbered by the std moves
                        # from the packed init (state col c = field idx c-4)
                        nc.vector.tensor_copy(ybig[0:8, 4:20],
                                              consts[0:8, 0:16])
                        nc.vector.tensor_copy(ybig[32:40, 84:100],
                                              consts[32:40, 80:96])

                # ONE custom FIR3 instruction per step: streams start at the
                # slot's scalar prefix (warm-up latches s1/s2/kappa), outputs
                # cover cols [4,100) = core+margins of all chunk groups.
                nc.vector._custom_dve(
                    fir3,
                    out=ybig[:, no + 4:no + 100],
                    in0=cur[:, co:co + 100],
                    in1=cur[:, co + 1:co + 101])

                j = t % tc_chunk
                if t % CG == CG - 1 or j == tc_chunk - 1:
                    # one ACT copy moves the last CG states (scaled basis),
                    # union cols [4,100) covering all chunk-group cores
                    g = CG if t % CG == CG - 1 else tc_chunk % CG
                    sb = (t - g + 1) % NB
                    j0 = j - g + 1
                    nc.scalar.copy(
                        av[:, j0:j0 + g, :],
                        yv[:, sb:sb + g, OFS + 1:OFS + 1 + AC],
                    )

                # per-group cell offsets: interior cores at acc idx [16,80),
                # chunk 0 (parts 0:8) at [0,64), chunk 15 (parts 32:40) at
                # [32,96).  The final window streams out in 25-step slices
                # as the copies land, so the unoverlapped tail is small.
                def _dma_out(c, ts_, te):
                    dst3 = out_dram[:, c * tc_chunk:(c + 1) * tc_chunk, :]
                    src3 = acc[:].rearrange("p (j n) -> p j n", n=AC)
                    nc.sync.dma_start(dst3[0:8, ts_:te, :],
                                      src3[0:8, ts_:te, 0:64])
                    nc.sync.dma_start(dst3[8:32, ts_:te, :],
                                      src3[8:32, ts_:te, 16:80])
                    nc.sync.dma_start(dst3[32:40, ts_:te, :],
                                      src3[32:40, ts_:te, 32:96])
                    nc.sync.dma_start(dst3[40:128, ts_:te, :],
                                      src3[40:128, ts_:te, 16:80])

                c = t // tc_chunk
                if c + 1 == n_chunks and j % 25 == 24:
                    _dma_out(c, j - 24, j + 1)
                elif j == tc_chunk - 1:
                    _dma_out(c, 0, tc_chunk)
                if j == tc_chunk - 1 and c + 1 < n_chunks:
                    acc = ap.tile([128, AC * tc_chunk], dt32, tag="acc")
                    av = acc[:].rearrange("p (j n) -> p j n", n=AC)

    nc.finalize()
    return nc


def _coeffs(params):
    d = params[:, 0].astype(np.float64)
    c = params[:, 1].astype(np.float64)
    alpha = DT * d / (DX * DX)
    beta = DT * c / (2.0 * DX)
    c0 = 1.0 - 2.0 * alpha
    cm = alpha + beta
    cp = alpha - beta
    sigma = 0.5 * (c0 + np.sqrt(c0 * c0 - 4.0 * cm * cp))
    return cm, cp, sigma


def _host_prep(init_conds, params):
    """Per-core packed input: shifted init + factored coeffs + shift selectors."""
    cm, cp, sigma = _coeffs(params)
    s1 = (cm / sigma).astype(np.float32)
    s2 = (cp / sigma).astype(np.float32)
    sig16 = (sigma ** H).astype(np.float32)

    # partition p = BLK[s]*8 + b; selector matrices route chunk halos (same
    # batch).  The boundary-source col slices are staged into the same fp16
    # tile on-device, so each direction is ONE merged selector matrix.
    w_psl = np.zeros((128, 128), np.float16)
    w_psr = np.zeros((128, 128), np.float16)
    for b in range(BL):
        for s in range(1, S):       # dest s left margin <- source s-1
            w_psl[BLK[s - 1] * 8 + b, BLK[s] * 8 + b] = 1.0
        for s in range(S - 1):      # dest s right margin <- source s+1
            w_psr[BLK[s + 1] * 8 + b, BLK[s] * 8 + b] = 1.0
    packh = np.zeros((128, 256), np.float16)
    packh[:, 0:128] = w_psl
    packh[:, 128:256] = w_psr

    in_maps = []
    for core in range(NCORES):
        sl = slice(core * BL, (core + 1) * BL)
        ic = np.ascontiguousarray(init_conds[sl]).astype(np.float32)
        pack = np.zeros((128, PACK), np.float32)
        icv = ic.reshape(BL, S, CW)
        for s in range(S):
            f0 = C0OF[s] - 1   # init-field idx of core cell 0 (state col-1)
            pack[BLK[s] * 8:BLK[s] * 8 + 8, f0:f0 + CW] = icv[:, s, :]
        pack[:, AC] = np.tile(s1[sl], S)
        pack[:, AC + 1] = np.tile(s2[sl], S)
        pack[:, AC + 2] = 1.0
        pack[:, AC + 3] = np.tile(s1[sl], S)
        pack[:, AC + 4] = np.tile(s2[sl], S)
        pack[:, AC + 5] = np.tile(sig16[sl], S)
        in_maps.append({"pack": pack, "packh": packh})
    return in_maps


def _unpermute(res):
    """[128, T, 64] block-major t-major -> [BL, N, T]."""
    tt = res.shape[1]
    r = res.reshape(S, BL, tt, CW)
    out = np.empty((BL, N, tt), res.dtype)
    for s in range(S):
        out[:, s * CW:(s + 1) * CW, :] = r[BLK[s]].transpose(0, 2, 1)
    return out


def kernel(init_conds, params):
    from concourse.bass_utils import run_bass_kernel_spmd

    if "nc" not in _CACHE:
        _CACHE["nc"] = _build(T, TC, H, M)
    nc = _CACHE["nc"]
    params = np.asarray(params)
    in_maps = _host_prep(np.asarray(init_conds), params)
    res = run_bass_kernel_spmd(nc, in_maps, list(range(NCORES)))
    outs = [_unpermute(np.asarray(res.results[c]["out"])) for c in range(NCORES)]
    out = np.concatenate(outs, axis=0)
    # host descale: stored state is y_t / sigma^(((t-1)%16)+1) for t>=1
    _, _, sigma = _coeffs(params)
    tt = np.arange(T)
    expo = np.where(tt == 0, 0, ((tt - 1) % H) + 1).astype(np.float64)
    fac = (sigma[:, None] ** expo[None, :]).astype(np.float32)  # [B, T]
    out *= fac[:, None, :]
    return out


# revision 36
# speedup vs baseline: 1.0092x; 1.0092x over previous
"""Trainium2 Bass kernel for nn_AdvDiffSolver: 1D advection-diffusion explicit Euler.

y_{t+1}[i] = c0*y[i] + cm*y[i-1] + cp*y[i+1]  (zero-padded boundaries), per-batch coeffs
  alpha = DT*d/DX^2, beta = DT*c/(2*DX);  c0 = 1-2a, cm = a+b, cp = a-b

FIR factorization (2 DVE ops/step): L = sigma*(1 + s1*E-)(1 + s2*E+) with
  sigma = (c0 + sqrt(c0^2 - 4*cm*cp))/2, s1 = cm/sigma, s2 = cp/sigma.
The device evolves the rescaled state within each 16-step window; a
tensor_scalar rescale by sigma^16 restores the basis at each margin refresh.
The per-window sigma^(phi+1) descale of the OUTPUT happens on the host.

Sharding: pure data parallel, 8 batches per core.  128 partitions = 16
spatial chunks x 8 batches.  Interior chunks (1..14) hold their 64-cell core
at cols [17,81) with 17-col halo margins both sides, refreshed every H=16
steps via PE shift-matmuls.  The two DOMAIN-BOUNDARY chunks are laid out
shifted so their Dirichlet pad cell falls on a column the step ops never
write: chunk 0 core at [1,65) (pad col 0; STT2 writes cols >= lo >= 1) and
chunk 15 core at [33,97) (pad col 97; STT2 writes cols < hi <= 97).  The pad
columns stay zero from init, so NO per-step boundary memsets are needed --
each step is exactly 2 fused scalar_tensor_tensor DVE ops.  Stale data
outside a boundary chunk's valid span decays inward 1 col/step and never
reaches the core within a refresh window.

Every 5 steps ONE ACT copy moves 5 states (union cols [1,97)) into the
[cell x t] accumulation buffer; each 125-step chunk is DMAd out with
per-chunk-group cell offsets so HBM only carries the 64 core cells.
Output leaves permuted [128, 64, T]; host unpermutes + descales.
"""

import numpy as np

B, N, T = 64, 1024, 1000
NCORES = 8
BL = B // NCORES      # 8 batches per core
S = 16                # spatial chunks per sample
CW = N // S           # 64 cells per chunk
M = 17                # margin cells each side (interior chunks)
H = 16                # margin refresh period (steps)
W = CW + 2 * M        # 98 tile cols
NB = 10               # state-slot rotation depth (multiple of copy group 5)
CG = 5                # steps per ACT accumulation copy
TC = 125              # time slices per accumulation chunk (8 chunks)
DX = 0.01
DT = 0.01
AC = W - 2            # 96: accumulated cols [1,97)
OFS = 3               # per-slot scalar prefix: [s1, s2, kappa]
WS = W + OFS          # 101 cols per state slot
# packed consts: init(96) | s1,s2,1.0 | s1,s2,sig16
PACK = AC + 6

# chunk s -> partition block (engine partition windows must start 32-aligned,
# so the boundary chunks sit at blocks 0 and 4: bases 0 and 32)
BLK = {}
for s in range(S):
    if s <= 3:
        BLK[s] = s
    elif s == 15:
        BLK[s] = 4
    else:
        BLK[s] = s + 1
# core column offset per chunk: boundary chunks shifted so the Dirichlet pad
# lands at col 0 (chunk 0) / col 97 (chunk 15)
C0OF = {s: (1 if s == 0 else (33 if s == 15 else M)) for s in range(S)}

_CACHE = {}


def _register_fir3():
    """Hand-authored custom DVE uop: one full Euler step per instruction.

    out[j] = kappa * (s1*y[j-1] + (1 + s1*s2)*y[j] + s2*y[j+1])
           = kappa * ((s1*y[j-1] + y[j]) + s2*(y[j+1] + s1*y[j]))

    Stream layout per row: cols [0,1,2] = s1, s2, kappa; col 3 = left pad;
    outputs at cols 4..  SRC_0 = center stream from col 0, SRC_1 = right
    stream from col 1.  The left tap is a one-element delay of SRC_0 via the
    stage-0 swap flop (BYPASS(A=CURR_SWAP_OUT, B=y_c) with swap_enable emits
    the previous element's y_c while latching the current one) --
    element-indexed state that travels with the stream, immune to issue
    bubbles.  Warm-up uops (write-suppressed) latch s1 -> swap@1+@2,
    s2 -> swap@4, kappa -> swap@7 from the first three stream elements (the
    BYPASS swap-latch stores the B operand), then one steady-shaped element
    primes the y-delay with the pad.  Outputs start at element 4.  No CONST
    operands, so there is no per-partition-scalar port penalty; kappa gives
    a free per-instruction output scale (1.0 in slots, sigma^H in yraw, so
    the refresh rebase costs nothing).
    """
    from dataclasses import dataclass
    from concourse import dve_ops as DO
    from concourse.dve_spec import Spec, Src0, Src1
    from concourse.dve_uop import (
        AluInp, AluOp, DelayInp, DveOpSpec, InpSel, OutPath, OutSel,
        Trigger, UopConfig, UopDpConfig,
    )

    for op in DO.OPS:
        if op.name == "FIR3B_ANT":
            return op

    EN, DIS = 1, 0
    A = AluInp

    def _dp(op, a, b, passthru=(), capture=None, swap=False):
        dp = UopDpConfig()
        dp.op = op
        dp.alu_src0 = a
        dp.alu_src1 = b
        dp.alu_out_enable = EN
        if swap:
            dp.swap_enable = EN
        for ln in passthru:
            dp.delay[ln] = DelayInp.PREV_DELAY
            dp.delay_enable[ln] = EN
        if capture is not None:
            dp.delay[capture] = DelayInp.PREV_ALU_OUT
            dp.delay_enable[capture] = EN
        return dp

    def _datapath():
        # lanes: d0 = y_c (SRC_0), d3 = y_r (SRC_1), d4 = m1 (captured)
        return [
            _dp(AluOp.BYPASS, A.CURR_SWAP_OUT, A.PREV_DELAY_0,
                passthru=(0, 3), swap=True),                  # y_l
            _dp(AluOp.MULTIPLY, A.PREV_ALU_OUT, A.CURR_SWAP_OUT,
                passthru=(0, 3)),                             # m1 = y_l*s1
            _dp(AluOp.MULTIPLY, A.PREV_DELAY_0, A.CURR_SWAP_OUT,
                passthru=(0, 3), capture=4),                  # m2 = y_c*s1
            _dp(AluOp.ADD, A.PREV_ALU_OUT, A.PREV_DELAY_3,
                passthru=(0, 4)),                             # a2 = m2+y_r
            _dp(AluOp.MULTIPLY, A.PREV_ALU_OUT, A.CURR_SWAP_OUT,
                passthru=(0, 4)),                             # m3 = a2*s2
            _dp(AluOp.ADD, A.PREV_ALU_OUT, A.PREV_DELAY_0,
                passthru=(4,)),                               # a3 = m3+y_c
            _dp(AluOp.ADD, A.PREV_ALU_OUT, A.PREV_DELAY_4),   # pre = a3+m1
            _dp(AluOp.MULTIPLY, A.PREV_ALU_OUT, A.CURR_SWAP_OUT),  # *kappa
        ]

    def _latch_datapath(latch_stages):
        st = [_dp(AluOp.BYPASS, A.CURR_SWAP_OUT, A.PREV_DELAY_0,
                  passthru=(0, 3), swap=True)]
        for i in range(1, 8):
            st.append(_dp(AluOp.BYPASS, A.PREV_ALU_OUT, A.PREV_DELAY_0,
                          passthru=(0, 3), swap=(i in latch_stages)))
        return st

    def _mk_uop(dp, repeat=0, nxt=(0, 0, 0), write=False):
        inp = [InpSel.ZERO] * 8
        inp_en = [DIS] * 8
        inp[1], inp_en[1] = InpSel.SRC_0, EN
        inp[4], inp_en[4] = InpSel.SRC_1, EN
        u = UopConfig()
        u.inp = inp
        u.inp_enable = inp_en
        u.require_inp0 = EN
        u.require_inp1 = EN
        u.repeat_count = repeat
        u.trigger = ((Trigger.COUNT, Trigger.NONE, Trigger.NONE) if repeat
                     else (Trigger.SRC_TENSOR_DONE, Trigger.NONE,
                           Trigger.NONE))
        u.next_uop = nxt
        u.datapath_config = dp
        if write:
            u.out_enable[OutPath.WR0_LO] = EN
            u.out[OutPath.WR0_LO] = OutSel.ALU_OUT
        else:
            for p in OutPath:
                u.out_enable[p] = DIS
        return u

    def _uops():
        return [
            _mk_uop(_latch_datapath({1, 2}), repeat=1, nxt=(1, 0, 0)),
            _mk_uop(_latch_datapath({4}), repeat=1, nxt=(2, 0, 0)),
            _mk_uop(_latch_datapath({7}), repeat=1, nxt=(3, 0, 0)),
            _mk_uop(_datapath(), repeat=1, nxt=(4, 0, 0)),
            _mk_uop(_datapath(), write=True),
        ]

    @dataclass(frozen=True)
    class HandDveOp:
        name: str
        spec: object
        subdim: bool

        def compile(self, ver):
            return DveOpSpec(
                name=self.name,
                opcode=DO.get_dve_sub_opcode(self.name),
                uops=_uops(),
                rd1_en=True,
            )

    spec = Spec(
        body=Src0 + Src1,
        reference=lambda in0, in1, s0, s1, imm2: in0 + in1,
    )
    op = HandDveOp(name="FIR3B_ANT", spec=spec, subdim=False)
    DO.OPS.append(op)
    DO._SUB_OPCODE_FOR_NAME[op.name] = DO._CUSTOM_DVE_ROW_BASE + len(DO.OPS) - 1
    DO.CUSTOM_DVE_SPECS[op.name] = spec
    return op


def _build(t_total, tc_chunk, h_refresh, m_margin):
    import concourse.bass as bass
    import concourse.bacc as bacc
    import concourse.mybir as mybir
    from concourse import tile

    dt32 = mybir.dt.float32
    w = CW + 2 * m_margin
    ws = w + OFS

    fir3 = _register_fir3()
    dt16 = mybir.dt.float16
    nc = bacc.Bacc(None, target_bir_lowering=False, debug=False)
    pack_in = nc.dram_tensor("pack", [128, PACK], dt32, kind="ExternalInput")
    packh_in = nc.dram_tensor("packh", [128, 256], dt16, kind="ExternalInput")
    # t-major output: per partition the DMA writes are fully sequential
    out_dram = nc.dram_tensor("out", [128, t_total, CW], dt32, kind="ExternalOutput")

    n_chunks = t_total // tc_chunk
    assert n_chunks * tc_chunk == t_total
    core_l = m_margin          # 17 (interior window anchor)
    core_r = m_margin + CW     # 81

    with tile.TileContext(nc) as tc:
        with (
            tc.tile_pool(name="state", bufs=1) as sp,
            tc.tile_pool(name="accp", bufs=2) as ap,
            tc.tile_pool(name="psum", bufs=2, space="PSUM") as pp,
        ):
            ybig = sp.tile([128, NB * ws], dt32, name="ybig", tag="ybig")
            yraw = sp.tile([128, ws], dt32, name="yraw", tag="yraw")
            consts = sp.tile([128, PACK], dt32, tag="consts")

            nc.sync.dma_start(consts[:], pack_in[:])
            consth = sp.tile([128, 256], dt16, tag="consth")
            nc.sync.dma_start(consth[:], packh_in[:])
            w_psl = consth[:, 0:128]
            w_psr = consth[:, 128:256]
            st = sp.tile([128, 34], dt16, tag="st")

            nc.vector.memset(ybig[:], 0.0)
            nc.vector.memset(yraw[:], 0.0)
            # init state (host pre-shifted per chunk group) into slot 0,
            # scalar triples [s1, s2, kappa] into every slot + yraw
            nc.vector.tensor_copy(ybig[:, OFS + 1:OFS + 1 + AC],
                                  consts[:, 0:AC])
            for b in range(NB):
                nc.vector.tensor_copy(ybig[:, b * ws:b * ws + 3],
                                      consts[:, AC:AC + 3])
            nc.vector.tensor_copy(yraw[:, 0:3], consts[:, AC + 3:AC + 6])
            acc = ap.tile([128, AC * tc_chunk], dt32, tag="acc")
            # time-major views: the grouped ACT copy is unit-stride on both
            # sides (slots are contiguous [nb][w] blocks; acc is [j][n])
            yv = ybig[:].rearrange("p (nb w) -> p nb w", w=ws)
            av = acc[:].rearrange("p (j n) -> p j n", n=AC)

            for t in range(1, t_total):
                cb = (t - 1) % NB
                nb = t % NB
                co = cb * ws  # cur col offset
                no = nb * ws  # nxt col offset
                cur = ybig
                if (t - 1) % h_refresh == 0:
                    psl = pp.tile([128, m_margin], dt32, tag="psl")
                    psr = pp.tile([128, m_margin], dt32, tag="psr")
                    # halo refresh: DVE stages the four source col-slices into
                    # one fp16 tile (std slice full-128, then aligned
                    # boundary-chunk overrides), then TWO fp16 partition-shift
                    # matmuls; all PSUM evacuation runs on DVE as plain
                    # copies (yraw's kappa = sigma^H does the rebase).
                    nc.vector.tensor_copy(st[:, 0:17], ybig[:, co + 67:co + 84])
                    nc.vector.tensor_copy(st[0:8, 0:17],
                                          ybig[0:8, co + 51:co + 68])
                    nc.vector.tensor_copy(st[:, 17:34], ybig[:, co + 20:co + 37])
                    nc.vector.tensor_copy(st[32:40, 17:34],
                                          ybig[32:40, co + 36:co + 53])
                    nc.tensor.matmul(psl[:], w_psl, st[:, 0:17],
                                     start=True, stop=True)
                    nc.tensor.matmul(psr[:], w_psr, st[:, 17:34],
                                     start=True, stop=True)
                    if t > 1:
                        nc.vector.tensor_copy(
                            yraw[:, 4:100], ybig[:, co + 4:co + 100])
                        # std margin moves (full 128) first, then aligned
                        # boundary-window moves, then boundary-core restores
                        # (psum rows are zero there and the std move clobbers)
                        nc.vector.tensor_copy(yraw[:, 3:20], psl[:])
                        nc.vector.tensor_copy(yraw[:, 84:101], psr[:])
                        nc.vector.tensor_copy(yraw[32:40, 19:36],
                                              psl[32:40, :])
                        nc.vector.tensor_copy(yraw[0:8, 68:85], psr[0:8, :])
                        nc.vector.tensor_copy(yraw[0:8, 4:20],
                                              ybig[0:8, co + 4:co + 20])
                        nc.vector.tensor_copy(yraw[32:40, 84:100],
                                              ybig[32:40, co + 84:co + 100])
                        cur = yraw
                        co = 0
                    else:
                        nc.vector.tensor_copy(ybig[:, 3:20], psl[:])
                        nc.vector.tensor_copy(ybig[:, 84:101], psr[:])
                        nc.vector.tensor_copy(ybig[32:40, 19:36],
                                              psl[32:40, :])
                        nc.vector.tensor_copy(ybig[0:8, 68:85], psr[0:8, :])
                        # restore boundary cores clobbered by the std moves
                        # from the packed init (state col c = field idx c-4)
                        nc.vector.tensor_copy(ybig[0:8, 4:20],
                                              consts[0:8, 0:16])
                        nc.vector.tensor_copy(ybig[32:40, 84:100],
                                              consts[32:40, 80:96])

                # ONE custom FIR3 instruction per step: streams start at the
                # slot's scalar prefix (warm-up latches s1/s2/kappa), outputs
                # cover cols [4,100) = core+margins of all chunk groups.
                nc.vector._custom_dve(
                    fir3,
                    out=ybig[:, no + 4:no + 100],
                    in0=cur[:, co:co + 100],
                    in1=cur[:, co + 1:co + 101])

                j = t % tc_chunk
                if t % CG == CG - 1 or j == tc_chunk - 1:
                    # one ACT copy moves the last CG states (scaled basis),
                    # union cols [4,100) covering all chunk-group cores
                    g = CG if t % CG == CG - 1 else tc_chunk % CG
                    sb = (t - g + 1) % NB
                    j0 = j - g + 1
                    nc.scalar.copy(
                        av[:, j0:j0 + g, :],
                        yv[:, sb:sb + g, OFS + 1:OFS + 1 + AC],
                    )

                # per-group cell offsets: interior cores at acc idx [16,80),
                # chunk 0 (parts 0:8) at [0,64), chunk 15 (parts 32:40) at
                # [32,96).  The final window streams out in 25-step slices
                # as the copies land, so the unoverlapped tail is small.
                def _dma_out(c, ts_, te):
                    dst3 = out_dram[:, c * tc_chunk:(c + 1) * tc_chunk, :]
                    src3 = acc[:].rearrange("p (j n) -> p j n", n=AC)
                    nc.sync.dma_start(dst3[0:8, ts_:te, :],
                                      src3[0:8, ts_:te, 0:64])
                    nc.sync.dma_start(dst3[8:32, ts_:te, :],
                                      src3[8:32, ts_:te, 16:80])
                    nc.sync.dma_start(dst3[32:40, ts_:te, :],
                                      src3[32:40, ts_:te, 32:96])
                    nc.sync.dma_start(dst3[40:128, ts_:te, :],
                                      src3[40:128, ts_:te, 16:80])

                c = t // tc_chunk
                if c + 1 == n_chunks and j % 25 == 24:
                    _dma_out(c, j - 24, j + 1)
                elif j == tc_chunk - 1:
                    _dma_out(c, 0, tc_chunk)
                if j == tc_chunk - 1 and c + 1 < n_chunks:
                    acc = ap.tile([128, AC * tc_chunk], dt32, tag="acc")
                    av = acc[:].rearrange("p (j n) -> p j n", n=AC)

    nc.finalize()
    return nc


def _coeffs(params):
    d = params[:, 0].astype(np.float64)
    c = params[:, 1].astype(np.float64)
    alpha = DT * d / (DX * DX)
    beta = DT * c / (2.0 * DX)
    c0 = 1.0 - 2.0 * alpha
    cm = alpha + beta
    cp = alpha - beta
    sigma = 0.5 * (c0 + np.sqrt(c0 * c0 - 4.0 * cm * cp))
    return cm, cp, sigma


def _host_prep(init_conds, params):
    """Per-core packed input: shifted init + factored coeffs + shift selectors."""
    cm, cp, sigma = _coeffs(params)
    s1 = (cm / sigma).astype(np.float32)
    s2 = (cp / sigma).astype(np.float32)
    sig16 = (sigma ** H).astype(np.float32)

    # partition p = BLK[s]*8 + b; selector matrices route chunk halos (same
    # batch).  The boundary-source col slices are staged into the same fp16
    # tile on-device, so each direction is ONE merged selector matrix.
    w_psl = np.zeros((128, 128), np.float16)
    w_psr = np.zeros((128, 128), np.float16)
    for b in range(BL):
        for s in range(1, S):       # dest s left margin <- source s-1
            w_psl[BLK[s - 1] * 8 + b, BLK[s] * 8 + b] = 1.0
        for s in range(S - 1):      # dest s right margin <- source s+1
            w_psr[BLK[s + 1] * 8 + b, BLK[s] * 8 + b] = 1.0
    packh = np.zeros((128, 256), np.float16)
    packh[:, 0:128] = w_psl
    packh[:, 128:256] = w_psr

    in_maps = []
    for core in range(NCORES):
        sl = slice(core * BL, (core + 1) * BL)
        ic = np.ascontiguousarray(init_conds[sl]).astype(np.float32)
        pack = np.zeros((128, PACK), np.float32)
        icv = ic.reshape(BL, S, CW)
        for s in range(S):
            f0 = C0OF[s] - 1   # init-field idx of core cell 0 (state col-1)
            pack[BLK[s] * 8:BLK[s] * 8 + 8, f0:f0 + CW] = icv[:, s, :]
        pack[:, AC] = np.tile(s1[sl], S)
        pack[:, AC + 1] = np.tile(s2[sl], S)
        pack[:, AC + 2] = 1.0
        pack[:, AC + 3] = np.tile(s1[sl], S)
        pack[:, AC + 4] = np.tile(s2[sl], S)
        pack[:, AC + 5] = np.tile(sig16[sl], S)
        in_maps.append({"pack": pack, "packh": packh})
    return in_maps


def _unpermute(res):
    """[128, T, 64] block-major t-major -> [BL, N, T]."""
    tt = res.shape[1]
    r = res.reshape(S, BL, tt, CW)
    out = np.empty((BL, N, tt), res.dtype)
    for s in range(S):
        out[:, s * CW:(s + 1) * CW, :] = r[BLK[s]].transpose(0, 2, 1)
    return out


def kernel(init_conds, params):
    from concourse.bass_utils import run_bass_kernel_spmd

    if "nc" not in _CACHE:
        _CACHE["nc"] = _build(T, TC, H, M)
    nc = _CACHE["nc"]
    params = np.asarray(params)
    in_maps = _host_prep(np.asarray(init_conds), params)
    res = run_bass_kernel_spmd(nc, in_maps, list(range(NCORES)))
    outs = [_unpermute(np.asarray(res.results[c]["out"])) for c in range(NCORES)]
    out = np.concatenate(outs, axis=0)
    # host descale: stored state is y_t / sigma^(((t-1)%16)+1) for t>=1
    _, _, sigma = _coeffs(params)
    tt = np.arange(T)
    expo = np.where(tt == 0, 0, ((tt - 1) % H) + 1).astype(np.float64)
    fac = (sigma[:, None] ** expo[None, :]).astype(np.float32)  # [B, T]
    out *= fac[:, None, :]
    return out
